# revision 61
# baseline (speedup 1.0000x reference)
"""Multi-head self-attention TRN2 kernel (8 NeuronCores, head-parallel).

Problem: x[L=4096, N=1, E=1024], w_qkv[3E, E], w_out[E, E], H=16 heads, DH=64.
Sharding: 2 heads per core (128 q/k/v dims). Each core computes its heads'
attention and a partial out-projection; host sums the 8 partials.

Per-core algorithm (bf16 matmuls, fp32 PSUM accumulation):
  qT[d,l] = wqT.T @ xT   (scale*log2e folded into wq on host -> S' = S*log2e)
  kT[d,l] = wkT.T @ xT
  V[l,d]  = xT.T @ wvT   per key tile
  For each query chunk (512 cols) and each key tile jt (128 rows):
    ST[j, i]  = kT[:,jt].T @ qT[:,chunk]   (both heads row-tiled on the PE)
    PT        = 2^ST'        (alternating engines: ScalarE Exp(scale=ln2) /
                              VectorE custom ANT_EXP2_BITS Schraudolph op;
                              no max subtraction: |S| <= ~5)
    O_h[.,i]  += V_h.T @ PT_h   (+ones col -> softmax denominator at row 64)
  OTn[d,i] = O_h[0:64] * broadcast(1/denom) ; out = OTn.T @ woT per l-tile.

The PV accumulation runs in MIXED PRECISION: key-tile pairs with even
index go through fp8e4 DoubleRow matmuls (one MM per head contracts BOTH
key tiles: P and V quantized to e4m3, ~1.9x per-pair PE speedup), odd
pairs stay bf16.  Full fp8 coverage gave 2.19% rel-l2 (over the 2e-2
gate, e4m3's 3-bit mantissa costs ~1.5% each via P and V); half coverage
lands at 1.62% with ~35us of PE saved.  The exp stream writes each pair's
probabilities straight to the matching dtype (ACT Exp output cast / DVE
bit-pattern op with per-dtype G/magic/alpha/H constants), so fp8 costs no
extra casts.

Steady state is paced jointly by the PE (~850ns/slot of matmul issue) and
the scores->exp->scores-buffer-reuse chain (~1850ns per 2 slots: st
double-buffer in PSUM, exp 1.15us on either engine); both land at
~920ns/slot -> ~299us.  PSUM is exactly full (4 banks scores, 2
projections, 2 O-accumulators), which blocks deeper score buffering.
"""

import sys
import os
import numpy as np

try:
    import concourse.bass as bass  # noqa: F401
except ImportError:
    sys.path.insert(0, "/opt/trn_rl_repo")

import ml_dtypes
import concourse.bass as bass
import concourse.mybir as mybir
import concourse.tile as tile
from concourse import bacc
from concourse.bass_utils import run_bass_kernel_spmd

BF16 = mybir.dt.bfloat16
F32 = mybir.dt.float32
I16 = mybir.dt.int16
FP8 = mybir.dt.float8e4
U8 = mybir.dt.uint8
AF = mybir.ActivationFunctionType

L, N, E, H = 4096, 1, 1024, 16
DH = E // H            # 64
P = 128                # partitions / dims per core (2 heads)
SCALE = DH ** -0.5
NCORES = 8
ET = E // P            # 8 contraction tiles for the projections

LOG2E = float(np.log2(np.e))
LN2 = float(np.log(2.0))
# Corrected-Schraudolph exp2 bit-pattern constants for the custom DVE op
# out = rint(G*x - (a*rho^2 + H)), rho = G*x - RN_G(G*x).  Two uses:
#  - bf16 bits (i16 out, G=128): ~0.88% rms vs 2^x
#  - fp8e4 bits (u8 out, G=8): ~2.8% rms (same scale as RNE e4m3 quant).
#    Negative bits saturate to 0 on HW (P=0 below 2^-7, harmless); NaN
#    bits (>=120) would need x>8, outside the score range.
EXPK16 = float(np.float32(1.5 * 2**30))
EXPA16 = 0.0023455
EXPH16 = -16251.896
EXPK8 = float(np.float32(1.5 * 2**26))
EXPA8 = 0.0023455 * 16.0
EXPH8 = -55.7435
# fp8 coverage: key-tile pairs with (pair % 2 == 0) run the PV in fp8e4
# DoubleRow (one matmul per head contracts both key tiles); odd pairs run
# bf16.  Half coverage keeps the final rel-l2 error ~1.6% (full fp8 was
# 2.19%, over the 2e-2 gate).


def _register_exp_op():
    """Register the custom DVE op computing float bit patterns of ~2^x.

    out = rint(G*x - (alpha*rho^2 + H)) with rho = G*x - RN_G(G*x) the signed
    fractional residual; with G = 2^mantissa_bits the integer out, viewed as
    that float type, is 2^round(x) * (1 + frac-correction) ~= 2^x.  G/magic/
    alpha/H are runtime operands, so one op serves both the bf16 (i16 out)
    and fp8e4 (u8 out) paths.  One 8-stage DVE pass per element, so the
    vector engine can take ~half the softmax exp stream off the scalar
    engine.  H rides in Src1 (full-shape constant tile): the [P,1]-broadcast
    Src1 encoding hangs trn2 hardware (probed), and all three scalar slots
    are taken.
    """
    from concourse import dve_ops
    from concourse.dve_spec import Spec, Src0, Src1, C0, C1, C2, lower
    from concourse.dve_uop import DveOpSpec
    from concourse.dve_ops import DveOp

    NAME = "ANT_EXP2_BITS"
    for op in dve_ops.OPS:
        if op.name == NAME:
            return op

    F = np.float32

    def _ref(in0, in1, s0, s1, imm2):
        p1 = (in0.astype(F) * F(s0)).astype(F)
        p2 = (p1 + F(s1)).astype(F)
        n0 = (p2 - F(s1)).astype(F)
        rho = (p1 - n0).astype(F)
        r = ((rho * rho).astype(F) * F(imm2)).astype(F)
        return (p1 - (r + in1.astype(F)).astype(F)).astype(F)

    p1 = Src0 * C0
    p2 = p1 + C1
    n0 = p2 - C1
    rho = p1 - n0
    r = (rho * rho) * C2
    body = p1 - (r + Src1)
    spec = Spec(body=body, reference=_ref)

    row = dve_ops._CUSTOM_DVE_ROW_BASE + len(dve_ops.OPS)
    assert row < 0x20
    dve_ops._SUB_OPCODE_FOR_NAME[NAME] = row
    shas = {}
    for ver in ("v3", "v4"):
        try:
            uops = lower(spec, ver=ver)
            shas[ver] = DveOpSpec(name=NAME, opcode=row, uops=uops,
                                  rd1_en=True).sha(ver)
        except Exception:
            pass
    op = DveOp(NAME, spec, subdim=False, uops_sha=shas)
    dve_ops.OPS.append(op)
    dve_ops.CUSTOM_DVE_SPECS[NAME] = spec
    return op


def build(nc, L=L):
    EXP_OP = _register_exp_op()
    LT = L // P            # key tiles
    CH = L // 512          # query chunks of 512
    CW = 512               # chunk width

    xT_d = nc.declare_dram_parameter("xT", [E, L], BF16, isOutput=False)
    wqT_d = nc.declare_dram_parameter("wqT", [E, P], BF16, isOutput=False)
    wkT_d = nc.declare_dram_parameter("wkT", [E, P], BF16, isOutput=False)
    wvT_d = nc.declare_dram_parameter("wvT", [E, P], BF16, isOutput=False)
    woT_d = nc.declare_dram_parameter("woT", [P, E], BF16, isOutput=False)
    out_d = nc.declare_dram_parameter("out", [L, E], BF16, isOutput=True)

    xT_t = xT_d.ap().rearrange("(t p) l -> p t l", p=P)
    wq_t = wqT_d.ap().rearrange("(t p) d -> p t d", p=P)
    wk_t = wkT_d.ap().rearrange("(t p) d -> p t d", p=P)
    wv_t = wvT_d.ap().rearrange("(t p) d -> p t d", p=P)
    out_t = out_d.ap().rearrange("(t p) f -> p t f", p=P)

    with tile.TileContext(nc) as tc:
        with (
            tc.tile_pool(name="persist", bufs=1) as sbp,
            tc.tile_pool(name="pt8", bufs=9) as sb_pt8,
            tc.tile_pool(name="pt16", bufs=16) as sb_pt16,
            tc.tile_pool(name="ob", bufs=2) as sb_ob,
            tc.tile_pool(name="misc", bufs=1) as sb_misc,
            tc.tile_pool(name="psbig", bufs=2, space="PSUM") as ps_big,
            tc.tile_pool(name="psone", bufs=2, space="PSUM") as ps_one,
            tc.tile_pool(name="pso", bufs=2, space="PSUM") as ps_o,
        ):
            # ---- persistent SBUF tiles + input DMAs ----
            # Each dma_start costs ~650 ns of serial issue time on the sync
            # engine, so batch aggressively with 3D access patterns: one DMA
            # per weight tensor, one per xT column block.
            wq_sb = sbp.tile([P, ET, P], BF16, tag="wq")
            wk_sb = sbp.tile([P, ET, P], BF16, tag="wk")
            wv_sb = sbp.tile([P, ET, P], BF16, tag="wv")

            # xT loaded in column blocks, one tile per block so each
            # projection chunk depends only on its own 1 MB of DMA traffic
            # instead of the full 8.4 MB transfer.  Block 0 is queued first
            # so the first projection can start ASAP.
            # chunk 0 as two separate half tiles (tile-granular dependency
            # tracking: the k projection's first 4 contraction tiles start
            # after ~0.5 MB of DMA instead of 1 MB); wk first (tiny)
            xtb0a = sbp.tile([P, ET // 4, CW], BF16, tag="xtb0a")
            xtb0a2 = sbp.tile([P, ET // 4, CW], BF16, tag="xtb0a2")
            xtb0b = sbp.tile([P, ET // 2, CW], BF16, tag="xtb0b")
            nc.sync.dma_start(out=wk_sb, in_=wk_t)
            nc.sync.dma_start(out=xtb0a, in_=xT_t[:, 0:ET // 4, 0:CW])
            nc.sync.dma_start(out=xtb0a2, in_=xT_t[:, ET // 4:ET // 2, 0:CW])
            nc.sync.dma_start(out=wq_sb, in_=wq_t)
            nc.sync.dma_start(out=xtb0b, in_=xT_t[:, ET // 2:, 0:CW])
            xT_sb = [(xtb0a, xtb0a2, xtb0b)]
            for lc in range(1, CH):
                xtb = sbp.tile([P, ET, CW], BF16, tag=f"xtb{lc}")
                nc.sync.dma_start(out=xtb,
                                  in_=xT_t[:, :, lc * CW:(lc + 1) * CW])
                xT_sb.append(xtb)
                if lc == 1:
                    nc.sync.dma_start(out=wv_sb, in_=wv_t)

            def xblk(lc, e):
                # xT contraction tile e of column block lc (block 0 is split
                # into quarter/quarter/half tiles so the first projection
                # matmuls start after 0.25 MB of DMA instead of 0.5 MB)
                if lc == 0:
                    a, a2, b = xT_sb[0]
                    if e < ET // 4:
                        return a[:, e, :]
                    if e < ET // 2:
                        return a2[:, e - ET // 4, :]
                    return b[:, e - ET // 2, :]
                return xT_sb[lc][:, e, :]
            wo_sb = sbp.tile([P, E], BF16, tag="wo")
            nc.sync.dma_start(out=wo_sb, in_=woT_d.ap())
            # full-shape Src1 constants for the DVE exp ([P,1] broadcast
            # hangs HW): H for the bf16-bits and fp8-bits variants
            hb16 = sbp.tile([P, 2 * CW], F32, tag="hb16")
            nc.vector.memset(hb16, float(np.float32(EXPH16)))
            hb8 = sbp.tile([P, 2 * CW], F32, tag="hb8")
            nc.vector.memset(hb8, float(np.float32(EXPH8)))

            # Dummy matmuls on a zeroed scratch tile during the initial DMA
            # wait: keeps the PE's HAM activity monitor busy so the clock is
            # already at 2.4 GHz when the first real projection lands.
            warm = sbp.tile([P, CW], BF16, tag="warm")
            nc.vector.memset(warm, 0.0)
            # tiny activation up front so the ~2.7us exp table load happens
            # during the initial DMA wait, not before the first real exp
            pre = sb_misc.tile([1, 8], BF16, tag="pre")
            nc.scalar.activation(out=pre, in_=warm[0:1, 0:8], func=AF.Exp)
            for _ in range(12):
                wp = ps_one.tile([P, CW], F32, tag="p1")
                nc.tensor.matmul(wp, lhsT=warm[:, 0:P], rhs=warm,
                                 start=True, stop=True)

            # V storage, one layout per PV flavor.  Even key-tile pairs: fp8
            # pair-tiles for the DoubleRow PV, per pair and head [128 j,
            # 2 k-slots, 80 cols] (64 V + 1 ones + 15 pad; the slot step must
            # be a multiple of 16 bytes).  Odd pairs: bf16 [V_A|1|V_B|1] as
            # in the all-bf16 kernel.  Column 64 (resp. augmented col) feeds
            # the softmax denominator row at PSUM partition 64.
            VW = 80
            NPAIR = LT // 2

            # fp8 coverage 8/16 pairs, alternating (rel-l2 1.62% vs the
            # 2e-2 gate).  10/16 coverage measured WORSE (301 vs 297us):
            # runs of consecutive fp8 pairs cluster the DVE's slower u8 exp
            # op and the scores->exp chain eats the PE saving.  Pair 0 must
            # be fp8 (its DR start covers PSUM rows [0:80)) and pair 15
            # bf16 (carries the stop flag on [0:65)).
            def pair_is_fp8(pp):
                return pp % 2 == 0

            V2A, V2B = {}, {}
            V_aug = {}
            for pp in range(NPAIR):
                if pair_is_fp8(pp):
                    for nm, fam in (("a", V2A), ("b", V2B)):
                        t = sbp.tile([P, 2, VW], FP8, tag=f"v8{nm}{pp}")
                        nc.vector.memset(t[:, :, DH:], 0.0)
                        nc.vector.memset(t[:, 0, DH:DH + 1], 1.0)
                        nc.vector.memset(t[:, 1, DH:DH + 1], 1.0)
                        fam[pp] = t
                else:
                    # [V_A|1|..pad..|V_B|1]: head B at column 96 so both
                    # halves are 32-aligned targets for the XBAR transpose
                    # (unaligned column offsets land displaced -- probed)
                    for lt in (2 * pp, 2 * pp + 1):
                        t = sbp.tile([P, 192], BF16, tag=f"va{lt}")
                        nc.vector.memset(t[:, DH:DH + 1], 1.0)
                        nc.vector.memset(t[:, 96 + DH:96 + DH + 1], 1.0)
                        V_aug[lt] = t

            qT = sbp.tile([P, L], BF16, tag="qT")
            kT = sbp.tile([P, L], BF16, tag="kT")
            OTn = sbp.tile([P, L], BF16, tag="otn")

            # ---- projections (mostly emitted as per-slot fillers inside the
            # attention stream so the ScalarE exp pipeline starts early) ----
            def proj_chunk(dst, w, lc):
                ps = ps_one.tile([P, CW], F32, tag="p1")
                for e in range(ET):
                    nc.tensor.matmul(
                        ps, lhsT=w[:, e, :], rhs=xblk(lc, e),
                        start=(e == 0), stop=(e == ET - 1))
                # PSUM->SBUF eviction on ACT (has slack); DVE is loaded with exp
                nc.scalar.copy(out=dst[:, lc * CW:(lc + 1) * CW], in_=ps)

            def proj_chunk_parts(dst, w, lc, nparts):
                # split one projection chunk into nparts small filler units so
                # a single attention slot never carries a ~2 us PE burst
                hold = {}
                step = ET // nparts

                def mk(i):
                    def go():
                        if i == 0:
                            ps = ps_one.tile([P, CW], F32, tag="p1")
                            hold["ps"] = ps
                        ps = hold["ps"]
                        for e in range(i * step, (i + 1) * step):
                            nc.tensor.matmul(
                                ps, lhsT=w[:, e, :], rhs=xblk(lc, e),
                                start=(e == 0), stop=(e == ET - 1))
                        if i == nparts - 1:
                            nc.scalar.copy(
                                out=dst[:, lc * CW:(lc + 1) * CW], in_=ps)
                    return go

                return [mk(i) for i in range(nparts)]

            def v_tile(lt):
                lc, off = lt // (CW // P), (lt % (CW // P)) * P

                def emit():
                    ps = ps_one.tile([P, P], F32, tag="p1")
                    for e in range(ET):
                        nc.tensor.matmul(
                            ps, lhsT=xblk(lc, e)[:, off:off + P],
                            rhs=wv_sb[:, e, :], start=(e == 0), stop=(e == ET - 1))
                    if pair_is_fp8(lt // 2):
                        nc.vector.tensor_copy(out=V2A[lt // 2][:, lt % 2, 0:DH],
                                              in_=ps[:, 0:DH])
                        nc.vector.tensor_copy(out=V2B[lt // 2][:, lt % 2, 0:DH],
                                              in_=ps[:, DH:2 * DH])
                    else:
                        nc.vector.tensor_copy(out=V_aug[lt][:, 0:DH],
                                              in_=ps[:, 0:DH])
                        nc.vector.tensor_copy(out=V_aug[lt][:, 96:96 + DH],
                                              in_=ps[:, DH:2 * DH])
                return emit

            # k-proj chunks first: chunk 0's scores sweep ALL key tiles
            # within its 32 slots, so kT chunk j must be written by slot ~4j
            fillers = []
            for lc in range(1, CH):
                fillers.append(lambda lc=lc: proj_chunk(kT, wk_sb, lc))
            for lt in range(LT):
                fillers.append(v_tile(lt))

            proj_chunk(kT, wk_sb, 0)
            proj_chunk(qT, wq_sb, 0)

            # ---- phase 2: attention ----
            def emit_scores(c, jt):
                st = ps_big.tile([P, 2 * CW], F32, tag="st")
                nc.tensor.matmul(
                    st[:, 0:CW], lhsT=kT[0:DH, jt * P:(jt + 1) * P],
                    rhs=qT[0:DH, c * CW:(c + 1) * CW], start=True, stop=True)
                nc.tensor.matmul(
                    st[:, CW:2 * CW], lhsT=kT[DH:P, jt * P:(jt + 1) * P],
                    rhs=qT[DH:P, c * CW:(c + 1) * CW], start=True, stop=True)
                return st

            def outproj_unit(c, lt, fc, stage):
                # out[l, f] for l-tile lt of chunk c, f columns [fc*512, +512)
                def emit():
                    glt = c * (CW // P) + lt
                    po = ps_one.tile([P, CW], F32, tag="p1")
                    nc.tensor.matmul(
                        po, lhsT=OTn[:, glt * P:(glt + 1) * P],
                        rhs=wo_sb[:, fc * CW:(fc + 1) * CW], start=True, stop=True)
                    # alternate eviction engine so back-to-back units are not
                    # serialized on one engine's PSUM->SBUF copies
                    eng = nc.vector.tensor_copy if (lt + fc) % 2 else nc.scalar.copy
                    eng(out=stage[:, lt, fc * CW:(fc + 1) * CW], in_=po)
                return emit

            def outproj_flush(c, stage, lt=None):
                def emit():
                    nt = CW // P
                    if lt is None:
                        nc.sync.dma_start(
                            out=out_t[:, c * nt:(c + 1) * nt, :], in_=stage)
                    else:
                        nc.sync.dma_start(
                            out=out_t[:, c * nt + lt:c * nt + lt + 1, :],
                            in_=stage[:, lt:lt + 1, :])
                return emit

            def emit_epilogue(c, o_a, o_b):
                # copy O to SBUF first (frees the PSUM banks so the next
                # chunk's PV can start; keeps the PE dense so the HAM clock
                # stays at 2.4 GHz), then normalize off the critical path.
                # Two per-head chains, interleaved so DVE / DMA / GpSimd steps
                # of head A overlap head B's.  Denominator rows live at
                # partition 64; custom-DVE ops can't shift partitions, so DMA
                # them to partition 0 first.
                oa_sb = sb_misc.tile([DH + 1, CW], F32, tag="oasb")
                ob_sb = sb_misc.tile([DH + 1, CW], F32, tag="obsb")
                dna = sb_misc.tile([1, CW], F32, tag="dna")
                dnb = sb_misc.tile([1, CW], F32, tag="dnb")
                raa = sb_misc.tile([1, CW], F32, tag="raa")
                rab = sb_misc.tile([1, CW], F32, tag="rab")
                bca = sb_misc.tile([DH, CW], F32, tag="bca")
                bcb = sb_misc.tile([DH, CW], F32, tag="bcb")
                nc.scalar.copy(out=oa_sb, in_=o_a[0:DH + 1])
                nc.sync.dma_start(out=dna, in_=oa_sb[DH:DH + 1, :])
                nc.scalar.copy(out=ob_sb, in_=o_b[0:DH + 1])
                nc.vector.reciprocal_approx_fast(out=raa, in_=dna)
                nc.sync.dma_start(out=dnb, in_=ob_sb[DH:DH + 1, :])
                nc.gpsimd.partition_broadcast(bca, raa)
                nc.vector.reciprocal_approx_fast(out=rab, in_=dnb)
                # muls stay on DVE: gpsimd pays ~7us Q7 reconfig per op-type
                # switch, which stalled the in-order PE queue behind outproj
                nc.vector.tensor_mul(
                    out=OTn[0:DH, c * CW:(c + 1) * CW],
                    in0=oa_sb[0:DH, :], in1=bca)
                nc.gpsimd.partition_broadcast(bcb, rab)
                otb = sb_misc.tile([DH, CW], BF16, tag="otb")
                nc.vector.tensor_mul(out=otb, in0=ob_sb[0:DH, :], in1=bcb)
                # partition shift 0:64 -> 64:128 via SBUF->SBUF DMA
                nc.sync.dma_start(out=OTn[DH:P, c * CW:(c + 1) * CW], in_=otb)
                stage = sb_ob.tile([P, CW // P, E], BF16, tag="ob")
                if c == CH - 1:
                    # tail: flush each l-tile as soon as its units finish
                    for lt in range(CW // P):
                        for fc in range(E // CW):
                            deferred.append(outproj_unit(c, lt, fc, stage))
                        deferred.append(outproj_flush(c, stage, lt))
                else:
                    for lt in range(CW // P):
                        for fc in range(E // CW):
                            deferred.append(outproj_unit(c, lt, fc, stage))
                    deferred.append(outproj_flush(c, stage))

            def mk_pv8(c, pp, ptp, od):
                # DoubleRow PV matmuls for chunk c, even key-tile PAIR pp:
                # one fp8 matmul per head contracts both key tiles (2x128
                # j's) of the pair at once.  O tiles allocated on first use
                # so their PSUM banks are claimed only when the deferred
                # stream starts.
                def emit():
                    if "t" not in od:
                        a = ps_o.tile([VW, CW], F32, tag="o")
                        b = ps_o.tile([VW, CW], F32, tag="o")
                        od["t"] = (a, b)
                    o_a, o_b = od["t"]
                    nc.tensor.matmul(
                        o_a, lhsT=V2A[pp], rhs=ptp[:, :, 0:CW],
                        perf_mode=mybir.MatmulPerfMode.DoubleRow,
                        start=(pp == 0), stop=False)
                    nc.tensor.matmul(
                        o_b, lhsT=V2B[pp], rhs=ptp[:, :, CW:2 * CW],
                        perf_mode=mybir.MatmulPerfMode.DoubleRow,
                        start=(pp == 0), stop=False)
                return emit

            def mk_pv16(c, pp, pts, od):
                # bf16 PV for odd pair pp: two matmuls per key tile as in the
                # all-bf16 kernel, accumulating into rows [0:65) of the same
                # PSUM group the fp8 pairs use (pair 0 is fp8 and starts the
                # [0:80) zero region, so these never touch unstarted rows).
                def emit():
                    o_a, o_b = od["t"]
                    for s, pt in enumerate(pts):
                        jt = 2 * pp + s
                        stop = (pp == NPAIR - 1 and s == 1)
                        nc.tensor.matmul(
                            o_a[0:DH + 1], lhsT=V_aug[jt][:, 0:DH + 1],
                            rhs=pt[:, 0:CW], start=False, stop=stop)
                        nc.tensor.matmul(
                            o_b[0:DH + 1], lhsT=V_aug[jt][:, 96:96 + DH + 1],
                            rhs=pt[:, CW:2 * CW], start=False, stop=stop)
                return emit

            # Software pipeline: PV(c, pp) executes ~2*DP slots after the
            # exps of its key-tile pair, so the exp stream never waits on V/K
            # production (which rides along as fillers in the early slots).
            # The last chunk drains the queue gradually so PE never bursts
            # while ACT idles.
            DP = 13 if CH > 1 else 0
            deferred = []
            pending = []          # (c, pp, pv-closure)
            ods = {c: {} for c in range(CH)}

            def pop_pv():
                pc, ppp, f = pending.pop(0)
                f()
                if ppp == NPAIR - 1:
                    emit_epilogue(pc, *ods[pc]["t"])

            # exp engine split: strict per-slot ACT/DVE alternation -- the
            # scores double-buffer (ps_big bufs=2) means exp(jt) must finish
            # within ~2 slot periods, so consecutive slots must land on
            # different engines or the PE stalls on the st WAR.  The op
            # variant follows the pair dtype (DVE: i16 bf16-bits / u8
            # fp8-bits; ACT: Exp to bf16 / fp8).  The last chunk's final
            # slots go to ACT so the deeper DVE queue can't stall the drain.
            def exp_on_dve(c, jt):
                return jt % 2 == 0

            st_cur = emit_scores(0, 0)
            ptp = None
            pts = []
            for c in range(CH):
                last = (c == CH - 1)
                for jt in range(LT):
                    # scores arrive as S' = S*log2e (folded into wq).  Even
                    # pairs: exp straight to fp8e4 (ACT Exp or DVE bit op)
                    # into one [P, 2, 2CW] tile whose k-slots feed the
                    # DoubleRow PV.  Odd pairs: bf16 tiles as in the all-bf16
                    # kernel.
                    fp8 = pair_is_fp8(jt // 2)
                    if fp8:
                        if jt % 2 == 0:
                            ptp = sb_pt8.tile([P, 2, 2 * CW], FP8, tag="pt8")
                        slot = ptp[:, jt % 2, :]
                        if exp_on_dve(c, jt):
                            nc.vector._custom_dve(
                                EXP_OP, out=slot.bitcast(U8), in0=st_cur,
                                in1=hb8, s0=8.0, s1=EXPK8,
                                imm2=float(np.float32(EXPA8)))
                        else:
                            nc.scalar.activation(out=slot, in_=st_cur,
                                                 func=AF.Exp, scale=LN2)
                    else:
                        if jt % 2 == 0:
                            pts = []
                        if exp_on_dve(c, jt):
                            pti = sb_pt16.tile([P, 2 * CW], I16, tag="pt16")
                            nc.vector._custom_dve(
                                EXP_OP, out=pti, in0=st_cur, in1=hb16,
                                s0=128.0, s1=EXPK16,
                                imm2=float(np.float32(EXPA16)))
                            pts.append(pti.bitcast(BF16))
                        else:
                            pt = sb_pt16.tile([P, 2 * CW], BF16, tag="pt16")
                            nc.scalar.activation(out=pt, in_=st_cur,
                                                 func=AF.Exp, scale=LN2)
                            pts.append(pt)
                    if jt < LT - 1:
                        st_next = emit_scores(c, jt + 1)
                    elif not last:
                        st_next = emit_scores(c + 1, 0)
                    if fillers:
                        fillers.pop(0)()
                    if jt % 2 == 1:
                        pp = jt // 2
                        mk = (mk_pv8(c, pp, ptp, ods[c]) if fp8
                              else mk_pv16(c, pp, pts, ods[c]))
                        pending.append((c, pp, mk))
                    if not last:
                        limit = DP
                    elif jt < LT - 6:
                        # drain gently but keep a >=2-pair lag so PV never
                        # waits synchronously on its exp
                        limit = max(2, DP - (jt + 1))
                    else:
                        limit = (LT - 1 - jt) // 2   # force-drain final slots
                    while len(pending) > limit:
                        pop_pv()
                    if deferred and jt % 2 == 0:
                        deferred.pop(0)()
                    if c + 1 < CH:
                        # q-projection for the next chunk as 8 single-MM
                        # parts spread over mid-chunk slots: 2-MM bursts at
                        # 4 slots measured slightly slower pipeline periods
                        if jt == 13:
                            qh = proj_chunk_parts(qT, wq_sb, c + 1, 8)
                            qh[0]()
                        elif jt in (15, 17, 19, 21, 23, 25, 27):
                            qh[(jt - 13) // 2]()
                    if jt < LT - 1 or not last:
                        st_cur = st_next
            while pending:
                pop_pv()
            for f in deferred:
                f()
    nc.finalize()
    return nc


_built = {}


def _get_nc(l=L):
    if l not in _built:
        nc = bacc.Bacc()
        _built[l] = build(nc, l)
    return _built[l]


def _prep_inputs(x, w_qkv, w_out, l=L):
    w_qkv = np.asarray(w_qkv, dtype=np.float32)
    w_out = np.asarray(w_out, dtype=np.float32)
    x2 = np.asarray(x, dtype=np.float32).reshape(l, E)
    xT = np.ascontiguousarray(x2.T).astype(ml_dtypes.bfloat16)
    wq, wk, wv = w_qkv[0:E], w_qkv[E:2 * E], w_qkv[2 * E:3 * E]
    in_maps = []
    for c in range(NCORES):
        d0 = c * P
        in_maps.append({
            "xT": xT,
            "wqT": np.ascontiguousarray(
                (wq[d0:d0 + P] * (SCALE * LOG2E)).T).astype(ml_dtypes.bfloat16),
            "wkT": np.ascontiguousarray(wk[d0:d0 + P].T).astype(ml_dtypes.bfloat16),
            "wvT": np.ascontiguousarray(wv[d0:d0 + P].T).astype(ml_dtypes.bfloat16),
            "woT": np.ascontiguousarray(
                w_out[:, d0:d0 + P].T).astype(ml_dtypes.bfloat16),
        })
    return in_maps


def _run(x, w_qkv, w_out, l=L, **kw):
    nc = _get_nc(l)
    in_maps = _prep_inputs(x, w_qkv, w_out, l)
    res = run_bass_kernel_spmd(nc, in_maps, core_ids=list(range(NCORES)), **kw)
    acc = np.zeros((l, E), dtype=np.float32)
    for r in res.results:
        acc += r["out"].astype(np.float32)
    return acc.reshape(l, N, E), res


def kernel(x, w_qkv, w_out):
    out, _ = _run(x, w_qkv, w_out)
    return out



# revision 62
# speedup vs baseline: 1.0125x; 1.0125x over previous
"""Multi-head self-attention TRN2 kernel (8 NeuronCores, head-parallel).

Problem: x[L=4096, N=1, E=1024], w_qkv[3E, E], w_out[E, E], H=16 heads, DH=64.
Sharding: 2 heads per core (128 q/k/v dims). Each core computes its heads'
attention and a partial out-projection; host sums the 8 partials.

Per-core algorithm (bf16 matmuls, fp32 PSUM accumulation):
  qT[d,l] = wqT.T @ xT   (scale*log2e folded into wq on host -> S' = S*log2e)
  kT[d,l] = wkT.T @ xT
  V[l,d]  = xT.T @ wvT   per key tile
  For each query chunk (512 cols) and each key tile jt (128 rows):
    ST[j, i]  = kT[:,jt].T @ qT[:,chunk]   (both heads row-tiled on the PE)
    PT        = 2^ST'        (alternating engines: ScalarE Exp(scale=ln2) /
                              VectorE custom ANT_EXP2_BITS Schraudolph op;
                              no max subtraction: |S| <= ~5)
    O_h[.,i]  += V_h.T @ PT_h   (+ones col -> softmax denominator at row 64)
  OTn[d,i] = O_h[0:64] * broadcast(1/denom) ; out = OTn.T @ woT per l-tile.

The PV accumulation runs in MIXED PRECISION: key-tile pairs with even
index go through fp8e4 DoubleRow matmuls (one MM per head contracts BOTH
key tiles: P and V quantized to e4m3, ~1.9x per-pair PE speedup), odd
pairs stay bf16.  Full fp8 coverage gave 2.19% rel-l2 (over the 2e-2
gate, e4m3's 3-bit mantissa costs ~1.5% each via P and V); half coverage
lands at 1.62% with ~35us of PE saved.  The exp stream writes each pair's
probabilities straight to the matching dtype (ACT Exp output cast / DVE
bit-pattern op with per-dtype G/magic/alpha/H constants), so fp8 costs no
extra casts.

Steady state is paced jointly by the PE (~850ns/slot of matmul issue) and
the scores->exp->scores-buffer-reuse chain (~1850ns per 2 slots: st
double-buffer in PSUM, exp 1.15us on either engine); both land at
~920ns/slot -> ~299us.  PSUM is exactly full (4 banks scores, 2
projections, 2 O-accumulators), which blocks deeper score buffering.
"""

import sys
import os
import numpy as np

try:
    import concourse.bass as bass  # noqa: F401
except ImportError:
    sys.path.insert(0, "/opt/trn_rl_repo")

import ml_dtypes
import concourse.bass as bass
import concourse.mybir as mybir
import concourse.tile as tile
from concourse import bacc
from concourse.bass_utils import run_bass_kernel_spmd

BF16 = mybir.dt.bfloat16
F32 = mybir.dt.float32
I16 = mybir.dt.int16
FP8 = mybir.dt.float8e4
U8 = mybir.dt.uint8
AF = mybir.ActivationFunctionType

L, N, E, H = 4096, 1, 1024, 16
DH = E // H            # 64
P = 128                # partitions / dims per core (2 heads)
SCALE = DH ** -0.5
NCORES = 8
ET = E // P            # 8 contraction tiles for the projections

LOG2E = float(np.log2(np.e))
LN2 = float(np.log(2.0))
# Corrected-Schraudolph exp2 bit-pattern constants for the custom DVE op
# out = rint(G*x - (a*rho^2 + H)), rho = G*x - RN_G(G*x).  Two uses:
#  - bf16 bits (i16 out, G=128): ~0.88% rms vs 2^x
#  - fp8e4 bits (u8 out, G=8): ~2.8% rms (same scale as RNE e4m3 quant).
#    Negative bits saturate to 0 on HW (P=0 below 2^-7, harmless); NaN
#    bits (>=120) would need x>8, outside the score range.
EXPK16 = float(np.float32(1.5 * 2**30))
EXPA16 = 0.0023455
EXPH16 = -16251.896
EXPK8 = float(np.float32(1.5 * 2**26))
EXPA8 = 0.0023455 * 16.0
EXPH8 = -55.7435
# fp8 coverage: key-tile pairs with (pair % 2 == 0) run the PV in fp8e4
# DoubleRow (one matmul per head contracts both key tiles); odd pairs run
# bf16.  Half coverage keeps the final rel-l2 error ~1.6% (full fp8 was
# 2.19%, over the 2e-2 gate).


def _register_exp_op():
    """Register the custom DVE op computing float bit patterns of ~2^x.

    out = rint(G*x - (alpha*rho^2 + H)) with rho = G*x - RN_G(G*x) the signed
    fractional residual; with G = 2^mantissa_bits the integer out, viewed as
    that float type, is 2^round(x) * (1 + frac-correction) ~= 2^x.  G/magic/
    alpha/H are runtime operands, so one op serves both the bf16 (i16 out)
    and fp8e4 (u8 out) paths.  One 8-stage DVE pass per element, so the
    vector engine can take ~half the softmax exp stream off the scalar
    engine.  H rides in Src1 (full-shape constant tile): the [P,1]-broadcast
    Src1 encoding hangs trn2 hardware (probed), and all three scalar slots
    are taken.
    """
    from concourse import dve_ops
    from concourse.dve_spec import Spec, Src0, Src1, C0, C1, C2, lower
    from concourse.dve_uop import DveOpSpec
    from concourse.dve_ops import DveOp

    NAME = "ANT_EXP2_BITS"
    for op in dve_ops.OPS:
        if op.name == NAME:
            return op

    F = np.float32

    def _ref(in0, in1, s0, s1, imm2):
        p1 = (in0.astype(F) * F(s0)).astype(F)
        p2 = (p1 + F(s1)).astype(F)
        n0 = (p2 - F(s1)).astype(F)
        rho = (p1 - n0).astype(F)
        r = ((rho * rho).astype(F) * F(imm2)).astype(F)
        return (p1 - (r + in1.astype(F)).astype(F)).astype(F)

    p1 = Src0 * C0
    p2 = p1 + C1
    n0 = p2 - C1
    rho = p1 - n0
    r = (rho * rho) * C2
    body = p1 - (r + Src1)
    spec = Spec(body=body, reference=_ref)

    row = dve_ops._CUSTOM_DVE_ROW_BASE + len(dve_ops.OPS)
    assert row < 0x20
    dve_ops._SUB_OPCODE_FOR_NAME[NAME] = row
    shas = {}
    for ver in ("v3", "v4"):
        try:
            uops = lower(spec, ver=ver)
            shas[ver] = DveOpSpec(name=NAME, opcode=row, uops=uops,
                                  rd1_en=True).sha(ver)
        except Exception:
            pass
    op = DveOp(NAME, spec, subdim=False, uops_sha=shas)
    dve_ops.OPS.append(op)
    dve_ops.CUSTOM_DVE_SPECS[NAME] = spec
    return op


def build(nc, L=L):
    EXP_OP = _register_exp_op()
    LT = L // P            # key tiles
    CH = L // 512          # query chunks of 512
    CW = 512               # chunk width

    xT_d = nc.declare_dram_parameter("xT", [E, L], BF16, isOutput=False)
    wqT_d = nc.declare_dram_parameter("wqT", [E, P], BF16, isOutput=False)
    wkT_d = nc.declare_dram_parameter("wkT", [E, P], BF16, isOutput=False)
    wvT_d = nc.declare_dram_parameter("wvT", [E, P], BF16, isOutput=False)
    woT_d = nc.declare_dram_parameter("woT", [P, E], BF16, isOutput=False)
    out_d = nc.declare_dram_parameter("out", [L, E], BF16, isOutput=True)

    xT_t = xT_d.ap().rearrange("(t p) l -> p t l", p=P)
    wq_t = wqT_d.ap().rearrange("(t p) d -> p t d", p=P)
    wk_t = wkT_d.ap().rearrange("(t p) d -> p t d", p=P)
    wv_t = wvT_d.ap().rearrange("(t p) d -> p t d", p=P)
    out_t = out_d.ap().rearrange("(t p) f -> p t f", p=P)

    with tile.TileContext(nc) as tc:
        with (
            tc.tile_pool(name="persist", bufs=1) as sbp,
            tc.tile_pool(name="pt8", bufs=9) as sb_pt8,
            tc.tile_pool(name="pt16", bufs=16) as sb_pt16,
            tc.tile_pool(name="ob", bufs=2) as sb_ob,
            tc.tile_pool(name="misc", bufs=1) as sb_misc,
            tc.tile_pool(name="psbig", bufs=2, space="PSUM") as ps_big,
            tc.tile_pool(name="psone", bufs=2, space="PSUM") as ps_one,
            tc.tile_pool(name="pso", bufs=2, space="PSUM") as ps_o,
        ):
            # ---- persistent SBUF tiles + input DMAs ----
            # Each dma_start costs ~650 ns of serial issue time on the sync
            # engine, so batch aggressively with 3D access patterns: one DMA
            # per weight tensor, one per xT column block.
            wq_sb = sbp.tile([P, ET, P], BF16, tag="wq")
            wk_sb = sbp.tile([P, ET, P], BF16, tag="wk")
            wv_sb = sbp.tile([P, ET, P], BF16, tag="wv")

            # xT loaded in column blocks, one tile per block so each
            # projection chunk depends only on its own 1 MB of DMA traffic
            # instead of the full 8.4 MB transfer.  Block 0 is queued first
            # so the first projection can start ASAP.
            # chunk 0 as two separate half tiles (tile-granular dependency
            # tracking: the k projection's first 4 contraction tiles start
            # after ~0.5 MB of DMA instead of 1 MB); wk first (tiny)
            xtb0a = sbp.tile([P, ET // 4, CW], BF16, tag="xtb0a")
            xtb0a2 = sbp.tile([P, ET // 4, CW], BF16, tag="xtb0a2")
            xtb0b = sbp.tile([P, ET // 2, CW], BF16, tag="xtb0b")
            nc.sync.dma_start(out=wk_sb, in_=wk_t)
            nc.sync.dma_start(out=xtb0a, in_=xT_t[:, 0:ET // 4, 0:CW])
            nc.sync.dma_start(out=xtb0a2, in_=xT_t[:, ET // 4:ET // 2, 0:CW])
            nc.sync.dma_start(out=wq_sb, in_=wq_t)
            nc.sync.dma_start(out=xtb0b, in_=xT_t[:, ET // 2:, 0:CW])
            xT_sb = [(xtb0a, xtb0a2, xtb0b)]
            for lc in range(1, CH):
                xtb = sbp.tile([P, ET, CW], BF16, tag=f"xtb{lc}")
                nc.sync.dma_start(out=xtb,
                                  in_=xT_t[:, :, lc * CW:(lc + 1) * CW])
                xT_sb.append(xtb)
                if lc == 1:
                    nc.sync.dma_start(out=wv_sb, in_=wv_t)

            def xblk(lc, e):
                # xT contraction tile e of column block lc (block 0 is split
                # into quarter/quarter/half tiles so the first projection
                # matmuls start after 0.25 MB of DMA instead of 0.5 MB)
                if lc == 0:
                    a, a2, b = xT_sb[0]
                    if e < ET // 4:
                        return a[:, e, :]
                    if e < ET // 2:
                        return a2[:, e - ET // 4, :]
                    return b[:, e - ET // 2, :]
                return xT_sb[lc][:, e, :]
            wo_sb = sbp.tile([P, E], BF16, tag="wo")
            nc.sync.dma_start(out=wo_sb, in_=woT_d.ap())
            # full-shape Src1 constants for the DVE exp ([P,1] broadcast
            # hangs HW): H for the bf16-bits and fp8-bits variants
            hb16 = sbp.tile([P, 2 * CW], F32, tag="hb16")
            nc.vector.memset(hb16, float(np.float32(EXPH16)))
            hb8 = sbp.tile([P, 2 * CW], F32, tag="hb8")
            nc.vector.memset(hb8, float(np.float32(EXPH8)))

            # Dummy matmuls on a zeroed scratch tile during the initial DMA
            # wait: keeps the PE's HAM activity monitor busy so the clock is
            # already at 2.4 GHz when the first real projection lands.
            warm = sbp.tile([P, CW], BF16, tag="warm")
            nc.vector.memset(warm, 0.0)
            # tiny activation up front so the ~2.7us exp table load happens
            # during the initial DMA wait, not before the first real exp
            pre = sb_misc.tile([1, 8], BF16, tag="pre")
            nc.scalar.activation(out=pre, in_=warm[0:1, 0:8], func=AF.Exp)
            for _ in range(12):
                wp = ps_one.tile([P, CW], F32, tag="p1")
                nc.tensor.matmul(wp, lhsT=warm[:, 0:P], rhs=warm,
                                 start=True, stop=True)

            # V storage, one layout per PV flavor.  Even key-tile pairs: fp8
            # pair-tiles for the DoubleRow PV, per pair and head [128 j,
            # 2 k-slots, 80 cols] (64 V + 1 ones + 15 pad; the slot step must
            # be a multiple of 16 bytes).  Odd pairs: bf16 [V_A|1|V_B|1] as
            # in the all-bf16 kernel.  Column 64 (resp. augmented col) feeds
            # the softmax denominator row at PSUM partition 64.
            VW = 80
            NPAIR = LT // 2

            # fp8 coverage 8/16 pairs, alternating (rel-l2 1.62% vs the
            # 2e-2 gate).  10/16 coverage measured WORSE (301 vs 297us):
            # runs of consecutive fp8 pairs cluster the DVE's slower u8 exp
            # op and the scores->exp chain eats the PE saving.  Pair 0 must
            # be fp8 (its DR start covers PSUM rows [0:80)) and pair 15
            # bf16 (carries the stop flag on [0:65)).
            def pair_is_fp8(pp):
                return pp % 2 == 0

            V2A, V2B = {}, {}
            V_aug = {}
            for pp in range(NPAIR):
                if pair_is_fp8(pp):
                    for nm, fam in (("a", V2A), ("b", V2B)):
                        t = sbp.tile([P, 2, VW], FP8, tag=f"v8{nm}{pp}")
                        nc.vector.memset(t[:, :, DH:], 0.0)
                        nc.vector.memset(t[:, 0, DH:DH + 1], 1.0)
                        nc.vector.memset(t[:, 1, DH:DH + 1], 1.0)
                        fam[pp] = t
                else:
                    # [V_A|1|..pad..|V_B|1]: head B at column 96 so both
                    # halves are 32-aligned targets for the XBAR transpose
                    # (unaligned column offsets land displaced -- probed)
                    for lt in (2 * pp, 2 * pp + 1):
                        t = sbp.tile([P, 192], BF16, tag=f"va{lt}")
                        nc.vector.memset(t[:, DH:DH + 1], 1.0)
                        nc.vector.memset(t[:, 96 + DH:96 + DH + 1], 1.0)
                        V_aug[lt] = t

            qT = sbp.tile([P, L], BF16, tag="qT")
            kT = sbp.tile([P, L], BF16, tag="kT")
            OTn = sbp.tile([P, L], BF16, tag="otn")

            # ---- projections (mostly emitted as per-slot fillers inside the
            # attention stream so the ScalarE exp pipeline starts early) ----
            def proj_chunk(dst, w, lc):
                ps = ps_one.tile([P, CW], F32, tag="p1")
                for e in range(ET):
                    nc.tensor.matmul(
                        ps, lhsT=w[:, e, :], rhs=xblk(lc, e),
                        start=(e == 0), stop=(e == ET - 1))
                # PSUM->SBUF eviction on ACT (has slack); DVE is loaded with exp
                nc.scalar.copy(out=dst[:, lc * CW:(lc + 1) * CW], in_=ps)

            def proj_chunk_parts(dst, w, lc, nparts):
                # split one projection chunk into nparts small filler units so
                # a single attention slot never carries a ~2 us PE burst
                hold = {}
                step = ET // nparts

                def mk(i):
                    def go():
                        if i == 0:
                            ps = ps_one.tile([P, CW], F32, tag="p1")
                            hold["ps"] = ps
                        ps = hold["ps"]
                        for e in range(i * step, (i + 1) * step):
                            nc.tensor.matmul(
                                ps, lhsT=w[:, e, :], rhs=xblk(lc, e),
                                start=(e == 0), stop=(e == ET - 1))
                        if i == nparts - 1:
                            nc.scalar.copy(
                                out=dst[:, lc * CW:(lc + 1) * CW], in_=ps)
                    return go

                return [mk(i) for i in range(nparts)]

            def v_tile(lt):
                lc, off = lt // (CW // P), (lt % (CW // P)) * P

                def emit():
                    ps = ps_one.tile([P, P], F32, tag="p1")
                    for e in range(ET):
                        nc.tensor.matmul(
                            ps, lhsT=xblk(lc, e)[:, off:off + P],
                            rhs=wv_sb[:, e, :], start=(e == 0), stop=(e == ET - 1))
                    if pair_is_fp8(lt // 2):
                        nc.vector.tensor_copy(out=V2A[lt // 2][:, lt % 2, 0:DH],
                                              in_=ps[:, 0:DH])
                        nc.vector.tensor_copy(out=V2B[lt // 2][:, lt % 2, 0:DH],
                                              in_=ps[:, DH:2 * DH])
                    else:
                        nc.vector.tensor_copy(out=V_aug[lt][:, 0:DH],
                                              in_=ps[:, 0:DH])
                        nc.vector.tensor_copy(out=V_aug[lt][:, 96:96 + DH],
                                              in_=ps[:, DH:2 * DH])
                return emit

            # k-proj chunks first: chunk 0's scores sweep ALL key tiles
            # within its 32 slots, so kT chunk j must be written by slot ~4j
            fillers = []
            for lc in range(1, CH):
                fillers.append(lambda lc=lc: proj_chunk(kT, wk_sb, lc))
            for lt in range(LT):
                fillers.append(v_tile(lt))

            proj_chunk(kT, wk_sb, 0)
            proj_chunk(qT, wq_sb, 0)

            # ---- phase 2: attention ----
            def emit_scores(c, jt):
                st = ps_big.tile([P, 2 * CW], F32, tag="st")
                nc.tensor.matmul(
                    st[:, 0:CW], lhsT=kT[0:DH, jt * P:(jt + 1) * P],
                    rhs=qT[0:DH, c * CW:(c + 1) * CW], start=True, stop=True)
                nc.tensor.matmul(
                    st[:, CW:2 * CW], lhsT=kT[DH:P, jt * P:(jt + 1) * P],
                    rhs=qT[DH:P, c * CW:(c + 1) * CW], start=True, stop=True)
                return st

            def outproj_unit(c, lt, fc, stage):
                # out[l, f] for l-tile lt of chunk c, f columns [fc*512, +512)
                def emit():
                    glt = c * (CW // P) + lt
                    po = ps_one.tile([P, CW], F32, tag="p1")
                    nc.tensor.matmul(
                        po, lhsT=OTn[:, glt * P:(glt + 1) * P],
                        rhs=wo_sb[:, fc * CW:(fc + 1) * CW], start=True, stop=True)
                    # alternate eviction engine so back-to-back units are not
                    # serialized on one engine's PSUM->SBUF copies
                    eng = nc.vector.tensor_copy if (lt + fc) % 2 else nc.scalar.copy
                    eng(out=stage[:, lt, fc * CW:(fc + 1) * CW], in_=po)
                return emit

            def outproj_flush(c, stage, lt=None):
                def emit():
                    nt = CW // P
                    if lt is None:
                        nc.sync.dma_start(
                            out=out_t[:, c * nt:(c + 1) * nt, :], in_=stage)
                    else:
                        nc.sync.dma_start(
                            out=out_t[:, c * nt + lt:c * nt + lt + 1, :],
                            in_=stage[:, lt:lt + 1, :])
                return emit

            def emit_epilogue(c, o_a, o_b):
                # copy O to SBUF first (frees the PSUM banks so the next
                # chunk's PV can start; keeps the PE dense so the HAM clock
                # stays at 2.4 GHz), then normalize off the critical path.
                # Two per-head chains, interleaved so DVE / DMA / GpSimd steps
                # of head A overlap head B's.  Denominator rows live at
                # partition 64; custom-DVE ops can't shift partitions, so DMA
                # them to partition 0 first.
                oa_sb = sb_misc.tile([DH + 1, CW], F32, tag="oasb")
                ob_sb = sb_misc.tile([DH + 1, CW], F32, tag="obsb")
                dna = sb_misc.tile([1, CW], F32, tag="dna")
                dnb = sb_misc.tile([1, CW], F32, tag="dnb")
                raa = sb_misc.tile([1, CW], F32, tag="raa")
                rab = sb_misc.tile([1, CW], F32, tag="rab")
                bca = sb_misc.tile([DH, CW], F32, tag="bca")
                bcb = sb_misc.tile([DH, CW], F32, tag="bcb")
                nc.scalar.copy(out=oa_sb, in_=o_a[0:DH + 1])
                nc.sync.dma_start(out=dna, in_=oa_sb[DH:DH + 1, :])
                nc.scalar.copy(out=ob_sb, in_=o_b[0:DH + 1])
                nc.vector.reciprocal_approx_fast(out=raa, in_=dna)
                nc.sync.dma_start(out=dnb, in_=ob_sb[DH:DH + 1, :])
                nc.gpsimd.partition_broadcast(bca, raa)
                nc.vector.reciprocal_approx_fast(out=rab, in_=dnb)
                # muls stay on DVE: gpsimd pays ~7us Q7 reconfig per op-type
                # switch, which stalled the in-order PE queue behind outproj
                nc.vector.tensor_mul(
                    out=OTn[0:DH, c * CW:(c + 1) * CW],
                    in0=oa_sb[0:DH, :], in1=bca)
                nc.gpsimd.partition_broadcast(bcb, rab)
                otb = sb_misc.tile([DH, CW], BF16, tag="otb")
                nc.vector.tensor_mul(out=otb, in0=ob_sb[0:DH, :], in1=bcb)
                # partition shift 0:64 -> 64:128 via SBUF->SBUF DMA
                nc.sync.dma_start(out=OTn[DH:P, c * CW:(c + 1) * CW], in_=otb)
                stage = sb_ob.tile([P, CW // P, E], BF16, tag="ob")
                if c == CH - 1:
                    # tail: flush each l-tile as soon as its units finish
                    for lt in range(CW // P):
                        for fc in range(E // CW):
                            deferred.append(outproj_unit(c, lt, fc, stage))
                        deferred.append(outproj_flush(c, stage, lt))
                else:
                    for lt in range(CW // P):
                        for fc in range(E // CW):
                            deferred.append(outproj_unit(c, lt, fc, stage))
                    deferred.append(outproj_flush(c, stage))

            def mk_pv8(c, pp, ptp, od):
                # DoubleRow PV matmuls for chunk c, even key-tile PAIR pp:
                # one fp8 matmul per head contracts both key tiles (2x128
                # j's) of the pair at once.  O tiles allocated on first use
                # so their PSUM banks are claimed only when the deferred
                # stream starts.
                def emit():
                    if "t" not in od:
                        a = ps_o.tile([VW, CW], F32, tag="o")
                        b = ps_o.tile([VW, CW], F32, tag="o")
                        od["t"] = (a, b)
                    o_a, o_b = od["t"]
                    nc.tensor.matmul(
                        o_a, lhsT=V2A[pp], rhs=ptp[:, :, 0:CW],
                        perf_mode=mybir.MatmulPerfMode.DoubleRow,
                        start=(pp == 0), stop=False)
                    nc.tensor.matmul(
                        o_b, lhsT=V2B[pp], rhs=ptp[:, :, CW:2 * CW],
                        perf_mode=mybir.MatmulPerfMode.DoubleRow,
                        start=(pp == 0), stop=False)
                return emit

            def mk_pv16(c, pp, pts, od):
                # bf16 PV for odd pair pp: two matmuls per key tile as in the
                # all-bf16 kernel, accumulating into rows [0:65) of the same
                # PSUM group the fp8 pairs use (pair 0 is fp8 and starts the
                # [0:80) zero region, so these never touch unstarted rows).
                def emit():
                    o_a, o_b = od["t"]
                    for s, pt in enumerate(pts):
                        jt = 2 * pp + s
                        stop = (pp == NPAIR - 1 and s == 1)
                        nc.tensor.matmul(
                            o_a[0:DH + 1], lhsT=V_aug[jt][:, 0:DH + 1],
                            rhs=pt[:, 0:CW], start=False, stop=stop)
                        nc.tensor.matmul(
                            o_b[0:DH + 1], lhsT=V_aug[jt][:, 96:96 + DH + 1],
                            rhs=pt[:, CW:2 * CW], start=False, stop=stop)
                return emit

            # Software pipeline: PV(c, pp) executes ~2*DP slots after the
            # exps of its key-tile pair, so the exp stream never waits on V/K
            # production (which rides along as fillers in the early slots).
            # The last chunk drains the queue gradually so PE never bursts
            # while ACT idles.
            DP = 12 if CH > 1 else 0
            deferred = []
            pending = []          # (c, pp, pv-closure)
            ods = {c: {} for c in range(CH)}

            def pop_pv():
                pc, ppp, f = pending.pop(0)
                f()
                if ppp == NPAIR - 1:
                    emit_epilogue(pc, *ods[pc]["t"])

            # exp engine split: strict per-slot ACT/DVE alternation -- the
            # scores double-buffer (ps_big bufs=2) means exp(jt) must finish
            # within ~2 slot periods, so consecutive slots must land on
            # different engines or the PE stalls on the st WAR.  The op
            # variant follows the pair dtype (DVE: i16 bf16-bits / u8
            # fp8-bits; ACT: Exp to bf16 / fp8).  The last chunk's final
            # slots go to ACT so the deeper DVE queue can't stall the drain.
            def exp_on_dve(c, jt):
                return jt % 2 == 0

            st_cur = emit_scores(0, 0)
            ptp = None
            pts = []
            for c in range(CH):
                last = (c == CH - 1)
                for jt in range(LT):
                    # scores arrive as S' = S*log2e (folded into wq).  Even
                    # pairs: exp straight to fp8e4 (ACT Exp or DVE bit op)
                    # into one [P, 2, 2CW] tile whose k-slots feed the
                    # DoubleRow PV.  Odd pairs: bf16 tiles as in the all-bf16
                    # kernel.
                    fp8 = pair_is_fp8(jt // 2)
                    if fp8:
                        if jt % 2 == 0:
                            ptp = sb_pt8.tile([P, 2, 2 * CW], FP8, tag="pt8")
                        slot = ptp[:, jt % 2, :]
                        if exp_on_dve(c, jt):
                            nc.vector._custom_dve(
                                EXP_OP, out=slot.bitcast(U8), in0=st_cur,
                                in1=hb8, s0=8.0, s1=EXPK8,
                                imm2=float(np.float32(EXPA8)))
                        else:
                            nc.scalar.activation(out=slot, in_=st_cur,
                                                 func=AF.Exp, scale=LN2)
                    else:
                        if jt % 2 == 0:
                            pts = []
                        if exp_on_dve(c, jt):
                            pti = sb_pt16.tile([P, 2 * CW], I16, tag="pt16")
                            nc.vector._custom_dve(
                                EXP_OP, out=pti, in0=st_cur, in1=hb16,
                                s0=128.0, s1=EXPK16,
                                imm2=float(np.float32(EXPA16)))
                            pts.append(pti.bitcast(BF16))
                        else:
                            pt = sb_pt16.tile([P, 2 * CW], BF16, tag="pt16")
                            nc.scalar.activation(out=pt, in_=st_cur,
                                                 func=AF.Exp, scale=LN2)
                            pts.append(pt)
                    if jt < LT - 1:
                        st_next = emit_scores(c, jt + 1)
                    elif not last:
                        st_next = emit_scores(c + 1, 0)
                    if fillers:
                        fillers.pop(0)()
                    if jt % 2 == 1:
                        pp = jt // 2
                        mk = (mk_pv8(c, pp, ptp, ods[c]) if fp8
                              else mk_pv16(c, pp, pts, ods[c]))
                        pending.append((c, pp, mk))
                    if not last:
                        limit = DP
                    elif jt < LT - 6:
                        # drain gently but keep a >=2-pair lag so PV never
                        # waits synchronously on its exp
                        limit = max(2, DP - (jt + 1))
                    else:
                        limit = (LT - 1 - jt) // 2   # force-drain final slots
                    while len(pending) > limit:
                        pop_pv()
                    if deferred and jt % 2 == 0:
                        deferred.pop(0)()
                    if c + 1 < CH:
                        # q-projection for the next chunk as 8 single-MM
                        # parts spread over mid-chunk slots: 2-MM bursts at
                        # 4 slots measured slightly slower pipeline periods
                        if jt == 13:
                            qh = proj_chunk_parts(qT, wq_sb, c + 1, 8)
                            qh[0]()
                        elif jt in (15, 17, 19, 21, 23, 25, 27):
                            qh[(jt - 13) // 2]()
                    if jt < LT - 1 or not last:
                        st_cur = st_next
            while pending:
                pop_pv()
            for f in deferred:
                f()
    nc.finalize()
    return nc


_built = {}


def _get_nc(l=L):
    if l not in _built:
        nc = bacc.Bacc()
        _built[l] = build(nc, l)
    return _built[l]


def _prep_inputs(x, w_qkv, w_out, l=L):
    w_qkv = np.asarray(w_qkv, dtype=np.float32)
    w_out = np.asarray(w_out, dtype=np.float32)
    x2 = np.asarray(x, dtype=np.float32).reshape(l, E)
    xT = np.ascontiguousarray(x2.T).astype(ml_dtypes.bfloat16)
    wq, wk, wv = w_qkv[0:E], w_qkv[E:2 * E], w_qkv[2 * E:3 * E]
    in_maps = []
    for c in range(NCORES):
        d0 = c * P
        in_maps.append({
            "xT": xT,
            "wqT": np.ascontiguousarray(
                (wq[d0:d0 + P] * (SCALE * LOG2E)).T).astype(ml_dtypes.bfloat16),
            "wkT": np.ascontiguousarray(wk[d0:d0 + P].T).astype(ml_dtypes.bfloat16),
            "wvT": np.ascontiguousarray(wv[d0:d0 + P].T).astype(ml_dtypes.bfloat16),
            "woT": np.ascontiguousarray(
                w_out[:, d0:d0 + P].T).astype(ml_dtypes.bfloat16),
        })
    return in_maps


def _run(x, w_qkv, w_out, l=L, **kw):
    nc = _get_nc(l)
    in_maps = _prep_inputs(x, w_qkv, w_out, l)
    res = run_bass_kernel_spmd(nc, in_maps, core_ids=list(range(NCORES)), **kw)
    acc = np.zeros((l, E), dtype=np.float32)
    for r in res.results:
        acc += r["out"].astype(np.float32)
    return acc.reshape(l, N, E), res


def kernel(x, w_qkv, w_out):
    out, _ = _run(x, w_qkv, w_out)
    return out

